# revision 4
# baseline (speedup 1.0000x reference)
import time
import numpy as np
import ml_dtypes

NC = 256
NC19 = 4864
B = 2
N = 8192
NSHARD = 4

_STAGES = {
    "sa1": dict(chans=(23, 64, 64, 128), cols=110592, ns=64),
    "sa2": dict(chans=(131, 128, 128, 256), cols=47104, ns=32),
    "sa3": dict(chans=(259, 128, 128, 256), cols=21504, ns=16),
    "sa4": dict(chans=(259, 128, 128, 256), cols=20480, ns=16),
}
_CACHE = {}
LAST_RUN_STATS = {}

BF16 = ml_dtypes.bfloat16


# ---------------- host-side numpy (validated bit-exact vs reference) ----------------

def fps_np(xyz, npoint):
    Npts = xyz.shape[0]
    mind = np.full((Npts,), 1e10, np.float32)
    far = 0
    out = np.empty((npoint,), np.int32)
    x, y, z = xyz[:, 0], xyz[:, 1], xyz[:, 2]
    for i in range(npoint):
        out[i] = far
        c = xyz[far]
        dx = x - c[0]
        dy = y - c[1]
        dz = z - c[2]
        d = (dx * dx + dy * dy) + dz * dz
        mind = np.minimum(mind, d)
        far = int(np.argmax(mind))
    return out


def pairwise_d2_np(a, b):
    a2 = np.sum(a * a, -1)
    b2 = np.sum(b * b, -1)
    return a2[:, None] + b2[None, :] - 2.0 * (a @ b.T)


def ball_query_np(radius, nsample, support, centers, block=1024):
    Nall = support.shape[0]
    M = centers.shape[0]
    out = np.empty((M, nsample), np.int32)
    for s in range(0, M, block):
        e = min(s + block, M)
        d2 = pairwise_d2_np(centers[s:e], support)
        score = np.where(d2 < radius * radius,
                         np.arange(Nall, dtype=np.int32)[None, :], Nall)
        part = np.partition(score, nsample - 1, axis=1)[:, :nsample]
        idx = np.sort(part, axis=1)
        idx = np.where(idx == Nall, idx[:, :1], idx)
        out[s:e] = np.where(idx == Nall, 0, idx)
    return out


def three_nn_np(q, s):
    d2 = pairwise_d2_np(q, s)
    idx = np.argsort(d2, axis=1, kind="stable")[:, :3]
    d = np.take_along_axis(d2, idx, axis=1)
    w = 1.0 / (d + 1e-8)
    w = w / np.sum(w, -1, keepdims=True)
    return idx.astype(np.int32), w.astype(np.float32)


def assemble_features(pointcloud, center_points, cue_points, matching,
                      matching_sem, floor_height, match_center):
    Npts = pointcloud.shape[0]
    xyz = pointcloud[:, :3]
    feats = pointcloud[:, 3:].T
    obj = np.concatenate([center_points, cue_points], axis=0)
    oh = np.zeros((18, 4864), np.float32)
    oh[matching_sem, np.arange(4864)] = 1.0
    center_feature = np.concatenate([
        (center_points[:, 2] - floor_height)[None, :],
        match_center[None, :], oh[:, :NC]], axis=0)
    cue_feature = np.concatenate([
        (cue_points[:, 2] - floor_height)[None, :],
        matching[None, :], oh[:, NC:]], axis=0)
    other = np.concatenate([feats, np.zeros((19, Npts), np.float32)], axis=0)
    features = np.concatenate([center_feature, cue_feature, other], axis=1)
    return xyz, obj, features


def mlp_host(x, layers):
    for (w, s, b) in layers:
        x = np.maximum((w * s[:, None]) @ x + b[:, None], 0.0)
    return x


# ---------------- device stage: dense MLP (3 layers) + windowed max-pool ----------------

def _build_stage(chans, cols, ns):
    from concourse import bacc, mybir
    import concourse.tile as tile
    K, C1, C2, C3 = chans
    TILE = 512
    nt = cols // TILE
    nw = TILE // ns
    M = cols // ns
    bf = mybir.dt.bfloat16
    f32 = mybir.dt.float32
    nc = bacc.Bacc(None, target_bir_lowering=False)
    x = nc.dram_tensor("x", [K, cols], bf, kind="ExternalInput")
    w1 = nc.dram_tensor("w1", [K, C1], bf, kind="ExternalInput")
    w2 = nc.dram_tensor("w2", [C1, C2], bf, kind="ExternalInput")
    w3 = nc.dram_tensor("w3", [C2, C3], bf, kind="ExternalInput")
    b1 = nc.dram_tensor("b1", [C1, 1], f32, kind="ExternalInput")
    b2 = nc.dram_tensor("b2", [C2, 1], f32, kind="ExternalInput")
    b3 = nc.dram_tensor("b3", [C3, 1], f32, kind="ExternalInput")
    out = nc.dram_tensor("out", [C3, M], f32, kind="ExternalOutput")
    kch = [(i, min(128, K - i)) for i in range(0, K, 128)]
    cch = [(i, min(128, C3 - i)) for i in range(0, C3, 128)]
    with tile.TileContext(nc) as tc:
        with (
            tc.tile_pool(name="singles", bufs=1) as singles,
            tc.tile_pool(name="xb", bufs=3) as xb,
            tc.tile_pool(name="yb", bufs=2) as yb,
            tc.tile_pool(name="pb", bufs=2) as pb,
            tc.tile_pool(name="ps12", bufs=2, space="PSUM") as ps12,
            tc.tile_pool(name="ps3", bufs=2, space="PSUM") as ps3,
        ):
            w1_t = []
            for (k0, kc) in kch:
                t = singles.tile([kc, C1], bf, name=f"w1_{k0}")
                nc.sync.dma_start(t[:], w1[k0:k0 + kc, :])
                w1_t.append(t)
            w2_t = singles.tile([C1, C2], bf)
            nc.sync.dma_start(w2_t[:], w2[:])
            w3_t = singles.tile([C2, C3], bf)
            nc.sync.dma_start(w3_t[:], w3[:])
            b1_t = singles.tile([C1, 1], f32)
            nc.sync.dma_start(b1_t[:], b1[:])
            b2_t = singles.tile([C2, 1], f32)
            nc.sync.dma_start(b2_t[:], b2[:])
            b3_t = []
            f_t = []
            for (c0, cc) in cch:
                t = singles.tile([cc, 1], f32, name=f"b3_{c0}")
                nc.sync.dma_start(t[:], b3[c0:c0 + cc, :])
                b3_t.append(t)
                f_t.append(singles.tile([cc, M], f32, name=f"f_{c0}"))
            for ti in range(nt):
                co = ti * TILE
                xt = []
                for (k0, kc) in kch:
                    t = xb.tile([kc, TILE], bf, name=f"xt_{k0}")
                    nc.sync.dma_start(t[:], x[k0:k0 + kc, co:co + TILE])
                    xt.append(t)
                z1 = ps12.tile([C1, TILE], f32, space="PSUM")
                for i in range(len(kch)):
                    nc.tensor.matmul(out=z1[:], lhsT=w1_t[i][:], rhs=xt[i][:],
                                     start=(i == 0), stop=(i == len(kch) - 1))
                y1 = yb.tile([C1, TILE], bf)
                nc.scalar.activation(y1[:], z1[:],
                                     mybir.ActivationFunctionType.Relu, bias=b1_t[:])
                z2 = ps12.tile([C2, TILE], f32, space="PSUM")
                nc.tensor.matmul(out=z2[:], lhsT=w2_t[:], rhs=y1[:],
                                 start=True, stop=True)
                y2 = yb.tile([C2, TILE], bf)
                nc.scalar.activation(y2[:], z2[:],
                                     mybir.ActivationFunctionType.Relu, bias=b2_t[:])
                for ci, (c0, cc) in enumerate(cch):
                    z3 = ps3.tile([cc, TILE], f32, space="PSUM")
                    nc.tensor.matmul(out=z3[:], lhsT=w3_t[:, c0:c0 + cc], rhs=y2[:],
                                     start=True, stop=True)
                    p = pb.tile([cc, nw], f32)
                    nc.vector.tensor_reduce(
                        out=p[:], in_=z3[:].rearrange("p (g n) -> p g n", n=ns),
                        axis=mybir.AxisListType.X, op=mybir.AluOpType.max)
                    nc.scalar.activation(f_t[ci][:, ti * nw:(ti + 1) * nw], p[:],
                                         mybir.ActivationFunctionType.Relu,
                                         bias=b3_t[ci][:])
            for ci, (c0, cc) in enumerate(cch):
                nc.sync.dma_start(out[c0:c0 + cc, :], f_t[ci][:])
    nc.compile()
    return nc


def _get_stage(name):
    if name not in _CACHE:
        sp = _STAGES[name]
        _CACHE[name] = _build_stage(sp["chans"], sp["cols"], sp["ns"])
    return _CACHE[name]


def _wmaps(layers):
    m = {}
    for i, (w, s, b) in enumerate(layers, 1):
        ws = (w.astype(np.float32) * s.astype(np.float32)[:, None])
        m[f"w{i}"] = np.ascontiguousarray(ws.T).astype(BF16)
        m[f"b{i}"] = np.ascontiguousarray(b.reshape(-1, 1)).astype(np.float32)
    return m


def _run_stage(name, x_full, layers):
    # x_full: list of B arrays [K, M_b*ns] fp32 -> returns list of B [C3, M_b] fp32
    from concourse.bass_utils import run_bass_kernel_spmd
    sp = _STAGES[name]
    cols = sp["cols"]
    nc = _get_stage(name)
    wm = _wmaps(layers)
    in_maps = []
    for b in range(B):
        xb16 = np.ascontiguousarray(x_full[b]).astype(BF16)
        for s in range(NSHARD):
            im = dict(wm)
            im["x"] = np.ascontiguousarray(xb16[:, s * cols:(s + 1) * cols])
            in_maps.append(im)
    t0 = time.time()
    res = run_bass_kernel_spmd(nc, in_maps, list(range(B * NSHARD)))
    LAST_RUN_STATS[name] = time.time() - t0
    LAST_RUN_STATS[name + "_in_maps"] = in_maps
    outs = [np.asarray(res.results[c]["out"], np.float32) for c in range(B * NSHARD)]
    return [np.concatenate(outs[b * NSHARD:(b + 1) * NSHARD], axis=1) for b in range(B)]


# ---------------- host orchestration ----------------

def _to_np(v):
    return np.asarray(v)


def _sa_host_prep(support_xyz, centers_xyz, feat, radius, nsample):
    # returns x [3+C, M*ns] fp32
    idx = ball_query_np(radius, nsample, support_xyz, centers_xyz)
    M, ns = idx.shape
    gx = (support_xyz[idx] - centers_xyz[:, None, :]) / radius  # [M,ns,3]
    gf = feat[:, idx]                                           # [C,M,ns]
    C = feat.shape[0]
    return np.concatenate([
        np.ascontiguousarray(gx.transpose(2, 0, 1)).reshape(3, M * ns),
        gf.reshape(C, M * ns)], axis=0)


def kernel(**inputs):
    pc = _to_np(inputs["pointcloud"]).astype(np.float32)
    cp = _to_np(inputs["center_points"]).astype(np.float32)
    cu = _to_np(inputs["cue_points"]).astype(np.float32)
    matching = _to_np(inputs["matching"]).astype(np.float32)
    matching_sem = _to_np(inputs["matching_sem"])
    floor_height = _to_np(inputs["floor_height"]).astype(np.float32)
    match_center = _to_np(inputs["match_center"]).astype(np.float32)
    sa2_inds = _to_np(inputs["sa2_inds"])
    sa3_inds = _to_np(inputs["sa3_inds"])
    sa4_inds = _to_np(inputs["sa4_inds"])
    params = {k: [(_to_np(w).astype(np.float32), _to_np(s).astype(np.float32),
                   _to_np(b).astype(np.float32)) for (w, s, b) in v]
              for k, v in inputs["params"].items()}

    t_host0 = time.time()
    per_b = []
    for b in range(B):
        xyz, obj, features = assemble_features(
            pc[b], cp[b], cu[b], matching[b], matching_sem[b],
            float(floor_height[b]), match_center[b])
        inds = fps_np(xyz, 2048)
        support1 = np.concatenate([obj, xyz], axis=0)       # [13056,3]
        xyz1 = np.concatenate([obj, xyz[inds]], axis=0)     # [6912,3]
        x1 = _sa_host_prep(support1, xyz1, features, 0.2, 64)
        per_b.append(dict(xyz=xyz, obj=obj, inds=inds, xyz1=xyz1, x1=x1))
    LAST_RUN_STATS["host_sa1_prep"] = time.time() - t_host0

    f1 = _run_stage("sa1", [d["x1"] for d in per_b], params["sa1"])  # [128,6912] x B

    x2 = []
    for b in range(B):
        d = per_b[b]
        xyz1 = d["xyz1"]
        new2 = np.concatenate([xyz1[:NC19], xyz1[NC19:][sa2_inds[b]]], axis=0)
        d["xyz2"] = new2
        x2.append(_sa_host_prep(xyz1, new2, f1[b], 0.4, 32))
    f2 = _run_stage("sa2", x2, params["sa2"])                        # [256,5888] x B

    x3 = []
    for b in range(B):
        d = per_b[b]
        xyz2 = d["xyz2"]
        new3 = np.concatenate([xyz2[:NC19], xyz2[NC19:][sa3_inds[b]]], axis=0)
        d["xyz3"] = new3
        x3.append(_sa_host_prep(xyz2, new3, f2[b], 0.8, 16))
    f3 = _run_stage("sa3", x3, params["sa3"])                        # [256,5376] x B

    x4 = []
    for b in range(B):
        d = per_b[b]
        xyz3 = d["xyz3"]
        new4 = np.concatenate([xyz3[:NC19], xyz3[NC19:][sa4_inds[b]]], axis=0)
        d["xyz4"] = new4
        x4.append(_sa_host_prep(xyz3, new4, f3[b], 1.2, 16))
    f4 = _run_stage("sa4", x4, params["sa4"])                        # [256,5120] x B

    t_fp0 = time.time()
    fp2_out = np.empty((B, 256, NC19), np.float32)
    for b in range(B):
        d = per_b[b]
        # fp1: only output cols 4864:5376 feed fp2 (per-column MLP => exact)
        q1 = d["xyz3"][NC19:NC19 + 512]
        s1 = d["xyz4"][NC19:]
        i1, w1 = three_nn_np(q1, s1)
        interp1 = np.einsum("cnk,nk->cn", f4[b][:, NC19:][:, i1], w1)
        xfp1 = np.concatenate([interp1, f3[b][:, NC19:NC19 + 512]], axis=0)
        fp1_out = mlp_host(xfp1, params["fp1"])                      # [256,512]
        # fp2
        q2 = d["xyz2"][:NC19]
        s2 = d["xyz3"][NC19:]
        i2, w2 = three_nn_np(q2, s2)
        interp2 = np.einsum("cnk,nk->cn", fp1_out[:, i2], w2)
        xfp2 = np.concatenate([interp2, f2[b][:, :NC19]], axis=0)
        fp2_out[b] = mlp_host(xfp2, params["fp2"])
    LAST_RUN_STATS["host_fp"] = time.time() - t_fp0

    xyz2_out = np.stack([d["obj"] for d in per_b], axis=0).astype(np.float32)
    inds_out = np.stack([d["inds"] for d in per_b], axis=0).astype(np.int32)
    return fp2_out, xyz2_out, inds_out
